# revision 56
# baseline (speedup 1.0000x reference)
"""Causal self-attention on 8 NeuronCores (Trainium2, Bass/Tile).

Sharding: core c handles batch b = c//2 and head-group hg = c%2
(8 of 16 heads = 512 of 1024 feature dims). W_qkv is split column-wise,
W_proj row-wise per head group; each core returns a partial [T, D]
projection output (bf16) and the host sums the two partials per batch.

All PE operands are bf16 (fp32r throttled the PE against the concurrent
exp ACTs and doubled LDWEIGHTS time); PSUM accumulation stays f32.

Causal masking is done ON the PE: a constant penalty C[p,x] =
-960*max(0, p-x) (= U.T @ L, U/L triangular constants) is accumulated
into the diagonal score tiles before the exp, so exp(SCALE*(s+C)) ~ 0
above the diagonal. No cross-engine mask hop in the score->exp->PV
chain (a DVE/gpsimd mask-mul was stalling the PV matmuls).

Per-core dataflow:
  xT = embds[b].T              [1024, 2048]  bf16 (host-transposed)
  qT/kT = Wq/Wk.T @ x.T        [512, 2048]   (head-dim major)
  v     = x @ Wv               [2048, 512]   (natural, + ones col per head)
  sT[j,i] = kT.T @ qT          per head, causal-skipped/shrunk tiles
  PT = exp(SCALE * (sT + C on diag))                bf16
  UT[e,i], denom[i] = [v|1].T @ PT                  (ones col -> denom)
  affinT = UT * (1/denom)      broadcast via K=8 bf16 matmul with E matrix
  outA = (ut0+ut1+ut2) @ Wp    dripped into the last attention chunk
  outB = ut3 @ Wp              tail
"""

import sys

for _p in ("/opt/trn_rl_repo",):
    if _p not in sys.path:
        sys.path.append(_p)

import ml_dtypes
import numpy as np

import concourse.bass as bass
import concourse.tile as tile
from concourse import bacc, mybir
from concourse.bass_utils import run_bass_kernel_spmd

F32 = mybir.dt.float32
BF16 = mybir.dt.bfloat16
EXP = mybir.ActivationFunctionType.Exp
COPY = mybir.ActivationFunctionType.Copy

B, T, D = 4, 2048, 1024
H, Dh = 16, 64
SCALE = float(D) ** -0.5
MPEN = -960.0     # per-step causal penalty; SCALE*MPEN = -30 per diag step
NCORES = 8
DL = 512          # local (per-core) feature width = 8 heads * 64
HL = 8            # local heads
NDC = D // 128    # 8 d-chunks
NEC = DL // 128   # 4 e-chunks (head pairs)
NTB = T // 512    # 4 t-blocks of 512
NTC = T // 128    # 16 t-chunks of 128
VPAIR = 192       # v_sb per-pair block: [v_even(64) | one | junk(63) | v_odd(64)]
VROW = NEC * VPAIR  # 640 cols per v_sb tile


def _build():
    nc = bacc.Bacc("TRN2", target_bir_lowering=False, debug=False,
                   num_devices=NCORES)

    xT = nc.declare_dram_parameter("xT", [D, T], BF16, isOutput=False)
    wq = nc.declare_dram_parameter("wq", [D, DL], BF16, isOutput=False)
    wk = nc.declare_dram_parameter("wk", [D, DL], BF16, isOutput=False)
    wv = nc.declare_dram_parameter("wv", [D, DL], BF16, isOutput=False)
    wp = nc.declare_dram_parameter("wp", [DL, D], BF16, isOutput=False)
    umat = nc.declare_dram_parameter("umat", [128, 128], BF16, isOutput=False)
    lmat = nc.declare_dram_parameter("lmat", [128, 128], BF16, isOutput=False)
    emat = nc.declare_dram_parameter("emat", [HL, DL], BF16, isOutput=False)
    outA = nc.declare_dram_parameter("outA", [T, D], BF16, isOutput=True)
    outB = nc.declare_dram_parameter("outB", [T, D], BF16, isOutput=True)

    with tile.TileContext(nc) as tc:
        _emit(nc, tc, xT, wq, wk, wv, wp, umat, lmat, emat, outA, outB)
    nc.compile()
    return nc


def _emit(nc, tc, xT, wq, wk, wv, wp, umat, lmat, emat, outA, outB):
    from contextlib import ExitStack

    ctx = ExitStack()
    with ctx:
        wqk_pool = ctx.enter_context(tc.tile_pool(name="wqk", bufs=4))
        qk_pool = ctx.enter_context(tc.tile_pool(name="qk", bufs=6))
        vsb_pool = ctx.enter_context(tc.tile_pool(name="vsb", bufs=NTC))
        ut_pool = ctx.enter_context(tc.tile_pool(name="ut", bufs=NEC))
        dn_pool = ctx.enter_context(tc.tile_pool(name="dn", bufs=1))
        dns_pool = ctx.enter_context(tc.tile_pool(name="dns", bufs=2))
        pt_pool = ctx.enter_context(tc.tile_pool(name="pt", bufs=5))
        cst_pool = ctx.enter_context(tc.tile_pool(name="cst", bufs=1))
        ps_pool = ctx.enter_context(tc.tile_pool(name="ps", bufs=1, space="PSUM"))

        # constants (DMAs issued later, after the startup-critical loads)
        um_sb = cst_pool.tile([128, 128], BF16, tag="um")
        lm_sb = cst_pool.tile([128, 128], BF16, tag="lm")
        em_sb = cst_pool.tile([HL, DL], BF16, tag="em")

        # persistent tiles
        ut_sb = [ut_pool.tile([128, T], BF16, tag="ut", name=f"ut{i}")
                 for i in range(NEC)]
        dn_sb = dn_pool.tile([HL, T], F32, tag="dn")
        rd_sb = dn_pool.tile([HL, T], F32, tag="rd")
        rdb_sb = dn_pool.tile([HL, T], BF16, tag="rdb")
        v_sb = [vsb_pool.tile([128, VROW], BF16, tag="vsb", name=f"vsb{i}")
                for i in range(NTC)]
        # garbage rows of dn would hit reciprocal before they are written;
        # keep them finite so 0*inf NaNs can't leak out of the R matmul
        nc.gpsimd.memset(dn_sb[:], 1.0)

        def ps_tile(tag, bufs):
            return ps_pool.tile([128, 512], F32, tag=tag, name=f"ps_{tag}",
                                bufs=bufs)

        wp_pool = ctx.enter_context(tc.tile_pool(name="wp", bufs=1))
        wp_big = wp_pool.tile([128, NEC * D], BF16, tag="wp", name="wp")

        with tc.tile_pool(name="xt", bufs=1) as xt_pool:
            # all 8 d-chunks of xT in one tile; chunk dc at cols [dc*T,(dc+1)*T)
            xt_big = xt_pool.tile([128, NDC * T], BF16, tag="xt", name="xt")
            xt_dr = xT[:].rearrange("(c p) t -> p c t", p=128)
            xt_v = xt_big[:].rearrange("p (c t) -> p c t", t=T)

            def xt_sb(dc):
                return xt_big[:, dc * T:(dc + 1) * T]

            qk_chunks = {}

            def qk_filler(ec):
                """Generator computing q/k chunks for `ec`; yields between
                small PE steps so it can be dripped into the attention loop
                as filler work that keeps the PE dense. kT is stored twice,
                zero-padded per head parity, so score matmuls run K=128.
                t-blocks go in order so the first block only needs the
                first quarter of xT (startup latency)."""
                wq_sb = wqk_pool.tile([128, NDC * 128], BF16, tag="wqk",
                                      name="wqec")
                wq_dr = wq[:, ec * 128:(ec + 1) * 128].rearrange(
                    "(c p) e -> p c e", p=128)
                wq_v = wq_sb[:].rearrange("p (c e) -> p c e", e=128)
                nc.sync.dma_start(wq_v[:, 0:8], wq_dr[:, 0:8])
                if ec == 0:
                    # DMA queues drain in issue order: the caller primes
                    # this generator so the wq DMA is issued before the
                    # (much larger) xT loads the first matmul also needs
                    yield "wq-primed"
                q_ec = qk_pool.tile([128, T], BF16, tag="qk", name="q_ec")
                kA = qk_pool.tile([128, T], BF16, tag="qk", name="kA")
                kB = qk_pool.tile([128, T], BF16, tag="qk", name="kB")
                nc.gpsimd.memset(kA[64:128, :], 0.0)
                nc.gpsimd.memset(kB[0:64, :], 0.0)
                qk_chunks[ec] = (q_ec, kA, kB)
                for (w_i, iskA) in ((0, False), (1, True)):
                    if iskA:   # defer the wk DMA until the k pass needs it
                        wk_sb = wqk_pool.tile([128, NDC * 128], BF16,
                                              tag="wqk", name="wkec")
                        nc.sync.dma_start(
                            wk_sb[:].rearrange("p (c e) -> p c e", e=128),
                            wk[:, ec * 128:(ec + 1) * 128].rearrange(
                                "(c p) e -> p c e", p=128))
                        w_sb = wk_sb
                    else:
                        w_sb = wq_sb
                    for tb in range(4):
                        ps = ps_tile("qkps", 2)
                        for dc in range(NDC):
                            nc.tensor.matmul(
                                ps, w_sb[:, dc * 128:(dc + 1) * 128],
                                xt_big[:, dc * T + tb * 512:
                                       dc * T + (tb + 1) * 512],
                                start=(dc == 0), stop=(dc == NDC - 1))
                            if dc % 2 == 1:
                                yield
                        sl = slice(tb * 512, (tb + 1) * 512)
                        if iskA:
                            nc.vector.tensor_copy(kA[0:64, sl], ps[0:64, :])
                            nc.vector.tensor_copy(kB[64:128, sl],
                                                  ps[64:128, :])
                        else:
                            nc.vector.tensor_copy(q_ec[:, sl], ps[:])
                        yield

            def normalize_recip(ec, tb, fast_cast=False):
                """1/denom (f32, DVE) + bf16 cast. Emitted a few slots
                before normalize_bcast so the PE's in-order queue never
                waits on the cast. The cast runs on idle gpsimd unless
                the bcast follows closely (fast_cast -> DVE)."""
                sl = slice(tb * 512, (tb + 1) * 512)
                nc.vector.reciprocal_approx_fast(rd_sb[:, sl], dn_sb[:, sl])
                if fast_cast:
                    nc.vector.tensor_copy(rdb_sb[:, sl], rd_sb[:, sl])
                else:
                    nc.gpsimd.tensor_copy(rdb_sb[:, sl], rd_sb[:, sl])

            def normalize_bcast(ec, tb, tail=False):
                """PE half: broadcast 1/denom across the 128 e-rows of
                chunk ec via the E matrix, then scale UT in place. In the
                tail, borrow the (idle) attention score PSUM tiles so the
                broadcast never sits in projB's qkps/utps buffer rotation
                (its DVE-mul reader would stall the next unit's matmul)."""
                sl = slice(tb * 512, (tb + 1) * 512)
                if tail:
                    ps_r = ps_pool.tile([128, 1024], F32, tag="stps",
                                        name="ps_stps", bufs=2)[:, 0:512]
                else:
                    ps_r = ps_tile("qkps", 2)
                nc.tensor.matmul(
                    ps_r[:], em_sb[:, ec * 128:(ec + 1) * 128],
                    rdb_sb[:, sl], start=True, stop=True)
                nc.vector.tensor_mul(
                    ut_sb[ec][:, sl], ut_sb[ec][:, sl], ps_r[:])

            def normalize(ec):
                for tb in range(NTB):
                    normalize_recip(ec, tb)
                    normalize_bcast(ec, tb)

            # ------------- phase A0: v = x @ Wv (+ dripped qk(0)) -------------
            with tc.tile_pool(name="wv", bufs=1) as wv_pool:
                filler0 = qk_filler(0)
                next(filler0)   # issue the wq DMA ahead of the xT loads
                sl = slice(0, T // 4)
                # first quarter split (2, 6) chunks: the first filler
                # matmuls only wait on wq + 128KB of xT
                nc.sync.dma_start(xt_v[0:128, 0:2, sl], xt_dr[0:128, 0:2, sl])
                nc.sync.dma_start(xt_v[0:128, 2:5, sl], xt_dr[0:128, 2:5, sl])
                nc.sync.dma_start(xt_v[0:128, 5:8, sl], xt_dr[0:128, 5:8, sl])
                wv_sb = wv_pool.tile([128, NDC * DL], BF16, tag="wv",
                                     name="wv")
                wv_v = wv_sb[:].rearrange("p (c e) -> p c e", e=DL)
                wv_dr = wv[:].rearrange("(c p) e -> p c e", p=128)
                nc.sync.dma_start(wv_v[:, 0:4], wv_dr[:, 0:4])
                nc.sync.dma_start(wv_v[:, 4:8], wv_dr[:, 4:8])
                for qt in range(1, 4):
                    sl = slice(qt * T // 4, (qt + 1) * T // 4)
                    nc.sync.dma_start(xt_v[:, :, sl], xt_dr[:, :, sl])
                # constants ride behind the startup-critical loads
                nc.sync.dma_start(um_sb[:], umat[:])
                nc.sync.dma_start(lm_sb[:], lmat[:])
                nc.sync.dma_start(em_sb[:], emat[:])

                for _ in range(6):   # pre-drip: PE work during the wv DMA
                    next(filler0, None)
                for tcn in range(NTC):
                    ps_v = ps_tile("utps", 2)
                    for dc in range(NDC):
                        nc.tensor.matmul(
                            ps_v[:],
                            xt_big[:, dc * T + tcn * 128:
                                   dc * T + (tcn + 1) * 128],
                            wv_sb[:, dc * DL:(dc + 1) * DL],
                            start=(dc == 0), stop=(dc == NDC - 1))
                    dst = v_sb[tcn][:].rearrange("p (e c) -> p e c", c=VPAIR)
                    src = ps_v[:].rearrange("p (e c) -> p e c", c=128)
                    nc.vector.tensor_copy(dst[:, :, 0:64], src[:, :, 0:64])
                    nc.vector.tensor_copy(dst[:, :, 128:192], src[:, :, 64:128])
                    nc.gpsimd.memset(dst[:, :, 64:65], 1.0)
                    nc.gpsimd.memset(dst[:, :, 65:128], 0.0)
                    next(filler0, None)
                for _ in filler0:
                    pass
            # wv pool released here

            # ------------- per e-chunk: attention + dripped filler work -------------
            def attention_chunk(ec, drip):
                q_ec, kA, kB = qk_chunks.pop(ec)
                slot = [0]
                for par in range(2):       # head parity within chunk
                    h = 2 * ec + par       # local head index
                    kpad = kA if par == 0 else kB
                    for ibp in range(2):   # i-block pair (2*ibp, 2*ibp+1)
                        ibl, ibr = 2 * ibp, 2 * ibp + 1
                        utl = ps_tile("utps", 2)
                        utr = ps_tile("utps", 2)
                        for jt in range(4 * ibr + 4):
                            drip(slot[0])
                            slot[0] += 1
                            dl = (jt // 4 == ibl)
                            skip_l = (jt // 4 > ibl)
                            dr = (jt // 4 == ibr)
                            cl = 128 * (jt - 4 * ibl) if dl else 0
                            cr = 128 * (jt - 4 * ibr) if dr else 0
                            c0 = 512 + cr if skip_l else cl
                            st_ps = ps_pool.tile([128, 1024], F32, tag="stps",
                                                 name="ps_stps", bufs=2)
                            kh_j = kpad[:, jt * 128:(jt + 1) * 128]
                            if not skip_l:
                                nc.tensor.matmul(
                                    st_ps[:, cl:512], kh_j,
                                    q_ec[:, ibl * 512 + cl:(ibl + 1) * 512],
                                    start=True, stop=not dl,
                                    skip_group_check=True)
                                if dl:
                                    nc.tensor.matmul(
                                        st_ps[:, cl:cl + 128], um_sb[:],
                                        lm_sb[:], start=False, stop=True,
                                        skip_group_check=True)
                            nc.tensor.matmul(
                                st_ps[:, 512 + cr:1024], kh_j,
                                q_ec[:, ibr * 512 + cr:(ibr + 1) * 512],
                                start=True, stop=not dr,
                                skip_group_check=True)
                            if dr:
                                nc.tensor.matmul(
                                    st_ps[:, 512 + cr:512 + cr + 128],
                                    um_sb[:], lm_sb[:], start=False,
                                    stop=True, skip_group_check=True)
                            pt_t = pt_pool.tile([128, 1024], BF16, tag="pt")
                            nc.scalar.activation(
                                pt_t[:, c0:1024], st_ps[:, c0:1024], EXP,
                                scale=SCALE)
                            # PV: [v|1].T @ PT -> UT rows + denom row
                            vt = v_sb[jt][:].rearrange(
                                "p (e c) -> p e c", c=VPAIR)[:, ec, :]
                            if par == 0:
                                lhs = vt[:, 0:65]       # M=65 -> rows 0..64
                                rsl = slice(0, 65)
                            else:
                                # [one|junk63|v_odd]: denom row 0, v 64..127
                                lhs = vt[:, 64:192]     # M=128
                                rsl = slice(0, 128)
                            if not skip_l:
                                nc.tensor.matmul(
                                    utl[rsl, cl:512], lhs, pt_t[:, cl:512],
                                    start=(jt == 0), stop=(jt == 4 * ibl + 3),
                                    skip_group_check=True)
                            nc.tensor.matmul(
                                utr[rsl, cr:512], lhs, pt_t[:, 512 + cr:1024],
                                start=(jt == 0), stop=(jt == 4 * ibr + 3),
                                skip_group_check=True)
                            for ib_d, ut_d in ((ibl, utl), (ibr, utr)):
                                if jt != 4 * ib_d + 3:
                                    continue
                                if par == 0:
                                    usrc, dsrc, r = (ut_d[0:64, :],
                                                     ut_d[64:65, :], 64)
                                    udst = ut_sb[ec][
                                        0:64, ib_d * 512:(ib_d + 1) * 512]
                                else:
                                    usrc, dsrc, r = (ut_d[64:128, :],
                                                     ut_d[0:1, :], 0)
                                    udst = ut_sb[ec][
                                        64:128, ib_d * 512:(ib_d + 1) * 512]
                                with tc.high_priority():
                                    nc.vector.tensor_copy(udst, usrc)
                                    # denom: same-partition copy + DMA repack
                                    stg = dns_pool.tile([128, 512], F32,
                                                        tag="dns",
                                                        name="dnstg")
                                    nc.vector.tensor_copy(stg[r:r + 1, :],
                                                          dsrc)
                                nc.sync.dma_start(
                                    dn_sb[h:h + 1,
                                          ib_d * 512:(ib_d + 1) * 512],
                                    stg[r:r + 1, :])

            RECIP_SLOTS = (0, 1, 2, 4)   # one (recip, cast) pair per slot

            for ec in range(NEC - 1):
                filler = qk_filler(ec + 1)
                if ec == NEC - 2:
                    # wp load rides the sync queue during chunk 2 so projA
                    # (dripped into chunk 3) never waits on it
                    nc.sync.dma_start(
                        wp_big[:].rearrange("p (c d) -> p c d", d=D),
                        wp[:].rearrange("(c p) d -> p c d", p=128))

                def drip(slot, ec=ec, filler=filler):
                    if ec > 0:
                        if slot in RECIP_SLOTS:
                            normalize_recip(ec - 1, RECIP_SLOTS.index(slot))
                        if slot % 4 == 3 and slot < 16:
                            normalize_bcast(ec - 1, slot // 4)
                    if slot % 4 != 3:   # 36 filler steps over 48 slots
                        next(filler, None)

                attention_chunk(ec, drip)
                for _ in filler:   # drain remaining qk(ec+1) work
                    pass
        # xt pool released here (before the last attention chunk)

        with tc.tile_pool(name="stage", bufs=4) as stage_pool:

            def proj_pass(ecs, out_t, use_act=False):
                """One projection pass accumulating a subset of e-chunks
                into its own partial output (summed on the host). Output
                rows are staged per 2-tcn group so each DMA moves 512KB
                (the sync queue dispatches ~1 DMA per 650ns — small DMAs
                throttle the tail). In the tail (use_act): stage casts
                alternate DVE/ACT and PSUM tiles alternate pool tags
                (utps is dead there) for a 4-deep matmul pipeline."""
                for tcg in range(NTC // 2):
                    st = stage_pool.tile([128, 2 * D], BF16, tag="st",
                                         name="stg")
                    for tc2 in range(2):
                        tcn = 2 * tcg + tc2
                        for ob in range(2):
                            if use_act and ob == 1:
                                ps_p = ps_tile("utps", 2)
                            else:
                                ps_p = ps_tile("qkps", 2)
                            for i, ecn in enumerate(ecs):
                                nc.tensor.matmul(
                                    ps_p[:],
                                    ut_sb[ecn][:,
                                               tcn * 128:(tcn + 1) * 128],
                                    wp_big[:, ecn * D + ob * 512:
                                           ecn * D + (ob + 1) * 512],
                                    start=(i == 0),
                                    stop=(i == len(ecs) - 1))
                            dst = st[:, tc2 * D + ob * 512:
                                     tc2 * D + (ob + 1) * 512]
                            if use_act and ob == 1:
                                # fixed buffer<->engine pairing: qkps tiles
                                # staged by DVE, utps by ACT — each PSUM
                                # rotation gated by exactly one engine
                                nc.scalar.activation(dst, ps_p[:], COPY)
                            else:
                                nc.vector.tensor_copy(dst, ps_p[:])
                            yield
                    if use_act and tcg == NTC // 2 - 1:
                        # last group: two half-size DMAs so the kernel's
                        # final wait is a 256KB transfer, not 512KB
                        # (finer splits lose more to the ~650ns/dispatch
                        # serial sync-queue cost than the transfer saves)
                        for tc2 in range(2):
                            nc.sync.dma_start(
                                out_t[tcg * 256 + tc2 * 128:
                                      tcg * 256 + (tc2 + 1) * 128, :],
                                st[:, tc2 * D:(tc2 + 1) * D])
                    else:
                        nc.sync.dma_start(
                            out_t[tcg * 256:(tcg + 1) * 256, :].rearrange(
                                "(c p) d -> p c d", p=128),
                            st[:].rearrange("p (c d) -> p c d", d=D))

            # last attention chunk: normalize(2) dripped first (recip/cast
            # early on DVE, bcast on filler-free slots), then proj pass A
            # (chunks 0-2) as filler
            projA = proj_pass((0, 1, 2), outA)
            # chunk-3 recips/bcasts run as soon as the matching dn rows
            # and ut3 col-ranges settle (par1 ib0/ib1/ib2 finish at slots
            # 27/31/43), so projB never waits on the normalize chain
            LATE_RECIPS = {30: 0, 33: 1, 45: 2}
            LATE_BCASTS = {35: 0, 39: 1}
            projB = proj_pass((3,), outB, use_act=True)

            def drip3(slot):
                if slot in RECIP_SLOTS:
                    normalize_recip(NEC - 2, RECIP_SLOTS.index(slot))
                if slot in LATE_RECIPS:
                    normalize_recip(NEC - 1, LATE_RECIPS[slot],
                                    fast_cast=True)
                if slot in LATE_BCASTS:
                    normalize_bcast(NEC - 1, LATE_BCASTS[slot])
                if slot in (43, 47):   # first projB units ride free slots
                    next(projB, None)  # (bcast(3,0) ran at slot 35)
                if slot % 4 == 3 and slot < 16:
                    normalize_bcast(NEC - 2, slot // 4)
                elif slot >= 4 and slot % 4 != 3:
                    next(projA, None)

            attention_chunk(NEC - 1, drip3)
            for _ in projA:
                pass
            # tail: bcast(tb+1) is emitted before projB's tb group so the
            # DVE mul overlaps the previous group's matmuls
            normalize_recip(NEC - 1, NTB - 1, fast_cast=True)
            normalize_bcast(NEC - 1, 2, tail=True)
            for _ in range(6):
                next(projB, None)
            normalize_bcast(NEC - 1, 3, tail=True)
            for _ in projB:
                pass



_NC_CACHE = None


def _get_nc():
    global _NC_CACHE
    if _NC_CACHE is None:
        _NC_CACHE = _build()
    return _NC_CACHE


def make_in_maps(embds, W_qkv, W_proj):
    embds = np.asarray(embds, dtype=np.float32)
    W_qkv = np.asarray(W_qkv, dtype=np.float32)
    W_proj = np.asarray(W_proj, dtype=np.float32)
    bf = ml_dtypes.bfloat16

    kk = np.arange(128)
    umat_np = (MPEN * (kk[:, None] <= kk[None, :])).astype(bf)   # [k, p]
    lmat_np = (1.0 * (kk[:, None] > kk[None, :])).astype(bf)     # [k, x]
    emat_np = np.kron(np.eye(HL), np.ones((1, Dh))).astype(bf)

    in_maps = []
    for c in range(NCORES):
        b, hg = c // 2, c % 2
        sl = slice(hg * DL, (hg + 1) * DL)
        in_maps.append({
            "xT": np.ascontiguousarray(embds[b].T).astype(bf),
            "wk": np.ascontiguousarray(W_qkv[:, 0 * D:1 * D][:, sl]).astype(bf),
            "wq": np.ascontiguousarray(W_qkv[:, 1 * D:2 * D][:, sl]).astype(bf),
            "wv": np.ascontiguousarray(W_qkv[:, 2 * D:3 * D][:, sl]).astype(bf),
            "wp": np.ascontiguousarray(W_proj[sl, :]).astype(bf),
            "umat": umat_np,
            "lmat": lmat_np,
            "emat": emat_np,
        })
    return in_maps


def gather_out(outs, b_proj):
    b_proj = np.asarray(b_proj, dtype=np.float32)
    full = np.empty((B, T, D), dtype=np.float32)
    for b in range(B):
        full[b] = outs[2 * b] + outs[2 * b + 1] + b_proj[None, :]
    return full


def kernel(embds, W_qkv, W_proj, b_proj):
    in_maps = make_in_maps(embds, W_qkv, W_proj)
    nc = _get_nc()
    res = run_bass_kernel_spmd(nc, in_maps, list(range(NCORES)))
    outs = [np.asarray(r["outA"], dtype=np.float32)
            + np.asarray(r["outB"], dtype=np.float32) for r in res.results]
    return gather_out(outs, b_proj)
